# revision 1
# baseline (speedup 1.0000x reference)
"""Tensor-parallel LlamaDecoderLayer forward on 8 Trainium2 NeuronCores. v2.

Sharding (per the TP hint): 4 q-heads + 1 kv-head per core; o/down
row-sharded; gate/up column-sharded over the (padded) intermediate dim.

v2 changes vs v1:
- all matmuls run in bf16 (fp32 matmul is 4 cyc/row on the PE, bf16 is 1)
- the RS+AG pair around norm2 is replaced by a single AllReduce of the
  o-proj partials, issued in four 512-token chunks that pipeline with
  attention + o-proj compute; every core then computes norm2 locally
- the final ReduceScatter is chunked the same way, and each core folds its
  own o-proj partial into the RS input so rs2 = mlp + o_sum arrives in one
  collective; the kernel output is just rs2 + x on the token shard
- gate/up/down weights are pre-slabbed on the host so weight DMA is
  contiguous; qkv + o weights stay resident in SBUF
"""

import numpy as np
import ml_dtypes

import concourse.bacc as bacc
import concourse.bass as bass
import concourse.mybir as mybir
import concourse.tile as tile
from concourse.bass_utils import run_bass_kernel_spmd
from concourse.masks import make_identity

AF = mybir.ActivationFunctionType
ALU = mybir.AluOpType
DT = mybir.dt
BF16 = ml_dtypes.bfloat16

FULL_CFG = dict(H=4096, S=1024, B=2, NQH=4, D=128, IC=1408, NC=8, EPS=1e-5)


def build_nc(cfg):
    H, S, B = cfg["H"], cfg["S"], cfg["B"]
    NQH, D, IC = cfg["NQH"], cfg["D"], cfg["IC"]
    NCORES, EPS = cfg["NC"], cfg["EPS"]
    T = B * S
    KT = H // 128           # hidden-dim k tiles
    FQK = NQH + 1           # q tiles + 1 k tile
    NF = FQK + 1            # + v tile
    KI = IC // 128          # intermediate k tiles (per-core shard)
    MH = H // 128           # output feature tiles
    TCH = 512               # token chunk (collective + matmul granularity)
    NCH = T // TCH          # 4 chunks; chunk ch = (batch ch//2, half ch%2)
    TSC = TCH // NCORES     # per-rank token shard within a chunk (64)
    TQ = TCH                # attention query chunk
    DIAG = TQ // 128        # diagonal (masked) kt blocks per query chunk
    SBK = S // 128          # seq kt blocks per batch
    HD = D // 2
    sm_scale = float(1.0 / np.sqrt(D))
    f32, bf16 = DT.float32, DT.bfloat16
    rg = [list(range(NCORES))]

    nc = bacc.Bacc("TRN2", target_bir_lowering=False, debug=False,
                   num_devices=NCORES)

    xt = nc.dram_tensor("xt", [H, T], bf16, kind="ExternalInput")
    xs = nc.dram_tensor("xs", [H, NCH * TSC], f32, kind="ExternalInput")
    wqkv = nc.dram_tensor("wqkv", [H, NF * 128], bf16, kind="ExternalInput")
    wo = nc.dram_tensor("wo", [NQH * D, H], bf16, kind="ExternalInput")
    wgu = nc.dram_tensor("wgu", [2 * KI, 128, KT, 128], bf16,
                         kind="ExternalInput")
    wdn = nc.dram_tensor("wdn", [MH, 128, KI, 128], bf16,
                         kind="ExternalInput")
    cs = nc.dram_tensor("cs", [D, T], f32, kind="ExternalInput")
    sn = nc.dram_tensor("sn", [D, T], f32, kind="ExternalInput")
    msk = nc.dram_tensor("msk", [DIAG, 128, TQ], bf16, kind="ExternalInput")
    out_t = nc.dram_tensor("out_t", [H, NCH * TSC], f32,
                           kind="ExternalOutput")

    wqkv_r = wqkv.ap().rearrange("(ko p) f -> p ko f", p=128)
    wo_r = wo.ap().rearrange("(ko p) f -> p ko f", p=128)

    with tile.TileContext(nc, num_cores=NCORES) as tc:
        with (
            tc.tile_pool(name="misc", bufs=1) as miscp,
            tc.tile_pool(name="small", bufs=2) as smallp,
            tc.tile_pool(name="dram", bufs=1, space="DRAM") as dramp,
            tc.tile_pool(name="ps", bufs=1, space="PSUM") as psp,
        ):
            ones_bf = miscp.tile([128, 1], bf16, tag="ones_bf")
            nc.gpsimd.memset(ones_bf, 1.0)
            eps_col = miscp.tile([128, 1], f32, tag="eps_col")
            nc.gpsimd.memset(eps_col, EPS)
            # ones row for PE-side partition broadcast (out = ones.T @ row).
            # gpsimd must stay free of compute: collective_compute blocks the
            # issuing engine queue until the ring completes, so any gpsimd op
            # queued after an in-flight collective would stall its consumers.
            ones_row = miscp.tile([1, 128], f32, tag="ones_row")
            nc.gpsimd.memset(ones_row, 1.0)

            def bcast(row, tag):
                ps_bc = psp.tile([128, row.shape[-1]], f32, tag="trbc",
                                 bufs=1, name=f"bc_{tag}")
                nc.tensor.matmul(ps_bc, ones_row, row, start=True, stop=True,
                                 skip_group_check=True)
                rb = smallp.tile([128, row.shape[-1]], f32, tag=tag)
                nc.scalar.copy(rb, ps_bc)
                return rb

            arin = [dramp.tile([H, TCH], bf16, tag=f"arin{c}",
                               name=f"arin{c}") for c in range(NCH)]
            arout = [dramp.tile([H, TCH], bf16, tag=f"arout{c}",
                                name=f"arout{c}", addr_space="Shared")
                     for c in range(NCH)]
            b2 = [dramp.tile([NCORES, H, TSC], bf16, tag=f"b2_{c}",
                             name=f"b2_{c}") for c in range(NCH)]
            rs2 = [dramp.tile([H, TSC], bf16, tag=f"rs2_{c}",
                              name=f"rs2_{c}") for c in range(NCH)]

            # ============ scope 1: qkv + attention + o-proj + AR ============
            with (
                tc.tile_pool(name="bigAB", bufs=1) as bigp,
                tc.tile_pool(name="strAB", bufs=4) as strp,
                tc.tile_pool(name="tmpAB", bufs=3) as tmpp,
            ):
                ident = bigp.tile([128, 128], f32, tag="ident")
                make_identity(nc, ident)
                cs_sb = bigp.tile([128, T], f32, tag="cs")
                nc.sync.dma_start(cs_sb, cs.ap())
                sn_sb = bigp.tile([128, T], f32, tag="sn")
                nc.sync.dma_start(sn_sb, sn.ap())
                msk_sb = bigp.tile([128, DIAG, TQ], bf16, tag="msk")
                for j in range(DIAG):
                    nc.sync.dma_start(msk_sb[:, j, :], msk.ap()[j])

                wqkv_sb = bigp.tile([128, KT, NF * 128], bf16, tag="wqkv")
                wo_sb = bigp.tile([128, NQH, H], bf16, tag="wo")
                qk_bf = bigp.tile([128, FQK, T], bf16, tag="qk")
                v_tok = bigp.tile([128, T // 128, 128], bf16, tag="vtok")
                attn_sb = bigp.tile([128, NQH, T], bf16, tag="attn")

                # ---- phase A: rmsnorm-folded qkv + rope (per token chunk) --
                def phase_a(tci):
                    t0 = tci * TCH
                    ps_qkv = [psp.tile([128, TCH], f32, tag="acc", bufs=6,
                                       name=f"ps_qkv{f}")
                              for f in range(NF)]
                    ps_ss = psp.tile([1, TCH], f32, tag="ss", bufs=1)
                    for k in range(KT):
                        if tci == 0:
                            nc.sync.dma_start(wqkv_sb[:, k, :],
                                              wqkv_r[:, k, :])
                        xtile = strp.tile([128, TCH], bf16, tag="x_in",
                                          bufs=4)
                        nc.sync.dma_start(
                            xtile, xt.ap()[k * 128:(k + 1) * 128,
                                           t0:t0 + TCH])
                        sq = strp.tile([128, TCH], bf16, tag="sq", bufs=3)
                        nc.scalar.square(sq, xtile)
                        nc.tensor.matmul(ps_ss, ones_bf, sq,
                                         start=(k == 0), stop=(k == KT - 1),
                                         skip_group_check=True)
                        for f in range(NF):
                            nc.tensor.matmul(
                                ps_qkv[f],
                                wqkv_sb[:, k, f * 128:(f + 1) * 128],
                                xtile, start=(k == 0), stop=(k == KT - 1),
                                skip_group_check=True)
                    ms = smallp.tile([1, TCH], f32, tag="ms")
                    nc.scalar.activation(ms, ps_ss, AF.Sqrt,
                                         bias=eps_col[0:1, :], scale=1.0 / H)
                    rr = smallp.tile([1, TCH], f32, tag="rr")
                    nc.vector.reciprocal(rr, ms)
                    rb = bcast(rr, "rb_a")
                    for f in range(FQK):
                        qf = tmpp.tile([128, TCH], f32, tag="qf", bufs=3)
                        nc.vector.tensor_tensor(qf, ps_qkv[f], rb, ALU.mult)
                        rot = tmpp.tile([128, TCH], f32, tag="rot", bufs=3)
                        nc.scalar.copy(rot[0:HD, :], qf[HD:D, :])
                        nc.scalar.copy(rot[HD:D, :], qf[0:HD, :])
                        nc.vector.tensor_tensor(rot, rot,
                                                sn_sb[:, t0:t0 + TCH],
                                                ALU.mult)
                        nc.vector.tensor_tensor(qf, qf,
                                                cs_sb[:, t0:t0 + TCH],
                                                ALU.mult)
                        nc.vector.tensor_tensor(qk_bf[:, f, t0:t0 + TCH],
                                                qf, rot, ALU.add)
                    vt = tmpp.tile([128, TCH], f32, tag="vt", bufs=2)
                    nc.vector.tensor_tensor(vt, ps_qkv[NF - 1], rb, ALU.mult)
                    ps_tr = psp.tile([128, TCH], f32, tag="trbc", bufs=1)
                    for j in range(TCH // 128):
                        nc.tensor.transpose(
                            ps_tr[:, j * 128:(j + 1) * 128],
                            vt[:, j * 128:(j + 1) * 128], ident)
                        nc.scalar.copy(v_tok[:, (t0 // 128) + j, :],
                                       ps_tr[:, j * 128:(j + 1) * 128])

                # ---- phase B+C per chunk: attention, o-proj, AllReduce ----
                def phase_bc(ch):
                    b, qc = ch // 2, ch % 2
                    qt0 = qc * TQ
                    nkt = (qt0 + TQ) // 128
                    diag0 = nkt - DIAG
                    for h in range(NQH):
                        ps_o = psp.tile([128, TQ], f32, tag="acc", bufs=6)
                        ps_den = psp.tile([1, TQ], f32, tag="ss", bufs=1)
                        for kt in range(nkt):
                            ps_s = psp.tile([128, TQ], f32, tag="acc",
                                            bufs=6)
                            nc.tensor.matmul(
                                ps_s,
                                qk_bf[:, NQH, b * S + kt * 128:
                                      b * S + (kt + 1) * 128],
                                qk_bf[:, h, b * S + qt0:b * S + qt0 + TQ],
                                start=True, stop=True,
                                skip_group_check=True)
                            e_sb = tmpp.tile([128, TQ], bf16, tag="e_sb",
                                             bufs=3)
                            nc.scalar.activation(e_sb, ps_s, AF.Exp,
                                                 scale=sm_scale)
                            if kt >= diag0:
                                nc.vector.tensor_tensor(
                                    e_sb, e_sb, msk_sb[:, kt - diag0, :],
                                    ALU.mult)
                            nc.tensor.matmul(
                                ps_o, v_tok[:, b * SBK + kt, :], e_sb,
                                start=(kt == 0), stop=(kt == nkt - 1),
                                skip_group_check=True)
                            nc.tensor.matmul(
                                ps_den, ones_bf, e_sb,
                                start=(kt == 0), stop=(kt == nkt - 1),
                                skip_group_check=True)
                        rden = smallp.tile([1, TQ], f32, tag="rden")
                        nc.vector.reciprocal(rden, ps_den)
                        rbd = bcast(rden, "rbd")
                        nc.vector.tensor_tensor(
                            attn_sb[:, h, b * S + qt0:b * S + qt0 + TQ],
                            ps_o, rbd, ALU.mult)

                    # o-proj for this chunk, then its AllReduce
                    for m in range(MH):
                        ps = psp.tile([128, TCH], f32, tag="acc", bufs=6)
                        for kh in range(NQH):
                            nc.tensor.matmul(
                                ps, wo_sb[:, kh, m * 128:(m + 1) * 128],
                                attn_sb[:, kh,
                                        b * S + qt0:b * S + qt0 + TCH],
                                start=(kh == 0), stop=(kh == NQH - 1))
                        ob = tmpp.tile([128, TCH], bf16, tag="o_bf", bufs=3)
                        # vector, not scalar: keeps the scalar queue clear so
                        # the post-AR arout reads can issue promptly
                        nc.vector.tensor_scalar_mul(ob, ps, 1.0)
                        nc.sync.dma_start(
                            arin[ch][m * 128:(m + 1) * 128, :], ob)
                    nc.gpsimd.collective_compute(
                        "AllReduce", ALU.add, replica_groups=rg,
                        ins=[arin[ch].opt()], outs=[arout[ch].opt()])

                # batch 0 first so AR0/AR1 fly while batch 1's qkv computes
                phase_a(0)
                phase_a(1)
                for kh in range(NQH):
                    nc.sync.dma_start(wo_sb[:, kh, :], wo_r[:, kh, :])
                phase_bc(0)
                phase_bc(1)
                phase_a(2)
                phase_a(3)
                phase_bc(2)
                phase_bc(3)

            # ============ scope 2: norm2 + MLP + RS per chunk ============
            with (
                tc.tile_pool(name="bigCD", bufs=1) as bigp2,
                tc.tile_pool(name="strCD", bufs=4) as strp2,
                tc.tile_pool(name="tmpCD", bufs=3) as tmpp2,
            ):
                for ch in range(NCH):
                    mi = bigp2.tile([128, KT, TCH], bf16, tag="mi", bufs=2)
                    ps_ss2 = psp.tile([1, TCH], f32, tag="ss", bufs=1)
                    for k in range(KT):
                        # arout reads ride the scalar queue: they wait on the
                        # AllReduce, and a sync-queue wait would head-of-line
                        # block every weight prefetch DMA queued behind it
                        ho = strp2.tile([128, TCH], bf16, tag="ho", bufs=6)
                        nc.scalar.dma_start(
                            ho, arout[ch][k * 128:(k + 1) * 128, :])
                        xa = strp2.tile([128, TCH], bf16, tag="xa", bufs=4)
                        nc.sync.dma_start(
                            xa, xt.ap()[k * 128:(k + 1) * 128,
                                        ch * TCH:(ch + 1) * TCH])
                        nc.vector.tensor_tensor(mi[:, k, :], ho, xa, ALU.add)
                        sqh = strp2.tile([128, TCH], bf16, tag="sqh", bufs=3)
                        nc.scalar.square(sqh, mi[:, k, :])
                        nc.tensor.matmul(ps_ss2, ones_bf, sqh,
                                         start=(k == 0), stop=(k == KT - 1),
                                         skip_group_check=True)
                    ms2 = smallp.tile([1, TCH], f32, tag="ms2")
                    nc.scalar.activation(ms2, ps_ss2, AF.Sqrt,
                                         bias=eps_col[0:1, :], scale=1.0 / H)
                    rr2 = smallp.tile([1, TCH], f32, tag="rr2")
                    nc.vector.reciprocal(rr2, ms2)
                    rb2 = bcast(rr2, "rb2")
                    for k in range(KT):
                        nc.vector.tensor_tensor(mi[:, k, :], mi[:, k, :],
                                                rb2, ALU.mult)

                    act = bigp2.tile([128, KI, TCH], bf16, tag="act", bufs=2)
                    for fi in range(KI):
                        wg = strp2.tile([128, KT, 128], bf16, tag="wg",
                                        bufs=3)
                        nc.sync.dma_start(wg, wgu.ap()[fi])
                        wu = strp2.tile([128, KT, 128], bf16, tag="wu",
                                        bufs=3)
                        nc.sync.dma_start(wu, wgu.ap()[KI + fi])
                        ps_g = psp.tile([128, TCH], f32, tag="acc", bufs=6)
                        ps_u = psp.tile([128, TCH], f32, tag="acc", bufs=6)
                        for k in range(KT):
                            nc.tensor.matmul(ps_g, wg[:, k, :], mi[:, k, :],
                                             start=(k == 0),
                                             stop=(k == KT - 1),
                                             skip_group_check=True)
                        for k in range(KT):
                            nc.tensor.matmul(ps_u, wu[:, k, :], mi[:, k, :],
                                             start=(k == 0),
                                             stop=(k == KT - 1),
                                             skip_group_check=True)
                        sg = tmpp2.tile([128, TCH], f32, tag="sg", bufs=2)
                        nc.scalar.activation(sg, ps_g, AF.Silu)
                        nc.vector.tensor_tensor(act[:, fi, :], sg, ps_u,
                                                ALU.mult)

                    # down-proj; fold in own o-proj partial so the RS wire
                    # carries mlp + o_sum in one pass
                    for m in range(MH):
                        wds = strp2.tile([128, KI, 128], bf16, tag="wd",
                                         bufs=4)
                        nc.sync.dma_start(wds, wdn.ap()[m])
                        ps = psp.tile([128, TCH], f32, tag="acc", bufs=6)
                        for k2 in range(KI):
                            nc.tensor.matmul(ps, wds[:, k2, :],
                                             act[:, k2, :],
                                             start=(k2 == 0),
                                             stop=(k2 == KI - 1))
                        opart = strp2.tile([128, TCH], bf16, tag="opart",
                                           bufs=3)
                        nc.sync.dma_start(
                            opart, arin[ch][m * 128:(m + 1) * 128, :])
                        ob2 = tmpp2.tile([128, TCH], bf16, tag="d_bf",
                                         bufs=3)
                        nc.vector.tensor_tensor(ob2, ps, opart, ALU.add)
                        nc.sync.dma_start(
                            b2[ch][:, m * 128:(m + 1) * 128, :]
                            .rearrange("r p t -> p r t"),
                            ob2.rearrange("p (r t) -> p r t", r=NCORES))
                    nc.gpsimd.collective_compute(
                        "ReduceScatter", ALU.add, replica_groups=rg,
                        ins=[b2[ch].opt()], outs=[rs2[ch].opt()])

                # ---- final: out = rs2 + x on own token shards ----
                for ch in range(NCH):
                    for k in range(KT):
                        rt = strp2.tile([128, TSC], bf16, tag="rs2_t",
                                        bufs=4)
                        nc.scalar.dma_start(
                            rt, rs2[ch][k * 128:(k + 1) * 128, :])
                        xst = strp2.tile([128, TSC], f32, tag="xs_t", bufs=4)
                        nc.sync.dma_start(
                            xst, xs.ap()[k * 128:(k + 1) * 128,
                                         ch * TSC:(ch + 1) * TSC])
                        ot = tmpp2.tile([128, TSC], f32, tag="out_t", bufs=4)
                        nc.vector.tensor_tensor(ot, rt, xst, ALU.add)
                        nc.sync.dma_start(
                            out_t.ap()[k * 128:(k + 1) * 128,
                                       ch * TSC:(ch + 1) * TSC], ot)

    nc.compile()
    return nc


def prepare_inputs(inputs, cfg):
    """Full np inputs -> per-core in_maps (host-side sharding/prep)."""
    H, S, B = cfg["H"], cfg["S"], cfg["B"]
    NQH, D, IC = cfg["NQH"], cfg["D"], cfg["IC"]
    NCORES = cfg["NC"]
    T = B * S
    KT = H // 128
    KI = IC // 128
    MH = H // 128
    TCH = 512
    NCH = T // TCH
    TSC = TCH // NCORES
    TQ = TCH
    DIAG = TQ // 128
    f4 = np.float32

    x = np.asarray(inputs["x"], f4).reshape(T, H)
    xt_f = np.ascontiguousarray(x.T)                    # [H, T] f32
    xt = xt_f.astype(BF16)
    cos = np.asarray(inputs["cos"], f4)                 # [S, D]
    sin = np.asarray(inputs["sin"], f4)
    cs = np.ascontiguousarray(np.tile(cos.T, (1, B)))   # [D, T]
    sn_s = sin.T.copy()
    sn_s[: D // 2] *= -1.0                              # sign-folded rot_half
    sn = np.ascontiguousarray(np.tile(sn_s, (1, B)))
    anw = np.asarray(inputs["attn_norm_w"], f4)
    fnw = np.asarray(inputs["ffn_norm_w"], f4)
    wq = np.asarray(inputs["wq"], f4) * anw[:, None]
    wk = np.asarray(inputs["wk"], f4) * anw[:, None]
    wv = np.asarray(inputs["wv"], f4) * anw[:, None]
    wo = np.asarray(inputs["wo"], f4)
    wg = np.asarray(inputs["w_gate"], f4) * fnw[:, None]
    wu = np.asarray(inputs["w_up"], f4) * fnw[:, None]
    wd = np.asarray(inputs["w_down"], f4)
    I_full = wg.shape[1]
    I_pad = NCORES * IC
    if I_pad > I_full:
        pad = I_pad - I_full
        wg = np.pad(wg, ((0, 0), (0, pad)))
        wu = np.pad(wu, ((0, 0), (0, pad)))
        wd = np.pad(wd, ((0, pad), (0, 0)))

    # causal masks, transposed layout: keep (j*128 + kt) <= q
    kt_i = np.arange(128)[:, None]
    q_i = np.arange(TQ)[None, :]
    msk = np.stack([(j * 128 + kt_i <= q_i) for j in range(DIAG)]
                   ).astype(BF16)

    # xs: own 64-token slices per chunk (f32 residual path)
    qd, kvd = NQH * D, D
    in_maps = []
    for c in range(NCORES):
        wqkv = np.concatenate([
            wq[:, c * qd:(c + 1) * qd],
            wk[:, c * kvd:(c + 1) * kvd],
            wv[:, c * kvd:(c + 1) * kvd]], axis=1).astype(BF16)
        gu = np.concatenate([
            wg[:, c * IC:(c + 1) * IC],
            wu[:, c * IC:(c + 1) * IC]], axis=1)       # [H, 2*IC]
        wgu_s = np.ascontiguousarray(
            gu.reshape(KT, 128, 2 * KI, 128).transpose(2, 1, 0, 3)
        ).astype(BF16)                                  # [2KI, 128, KT, 128]
        dn = wd[c * IC:(c + 1) * IC, :]                 # [IC, H]
        wdn_s = np.ascontiguousarray(
            dn.reshape(KI, 128, MH, 128).transpose(2, 1, 0, 3)
        ).astype(BF16)                                  # [MH, 128, KI, 128]
        xs = np.empty((H, NCH * TSC), f4)
        for ch in range(NCH):
            g0 = ch * TCH + c * TSC
            xs[:, ch * TSC:(ch + 1) * TSC] = xt_f[:, g0:g0 + TSC]
        in_maps.append({
            "xt": xt,
            "xs": np.ascontiguousarray(xs),
            "wqkv": np.ascontiguousarray(wqkv),
            "wo": np.ascontiguousarray(wo[c * qd:(c + 1) * qd, :]
                                       ).astype(BF16),
            "wgu": wgu_s,
            "wdn": wdn_s,
            "cs": cs, "sn": sn, "msk": msk,
        })
    return in_maps


def assemble_output(results, cfg):
    H, S, B, NCORES = cfg["H"], cfg["S"], cfg["B"], cfg["NC"]
    T = B * S
    TCH = 512
    NCH = T // TCH
    TSC = TCH // NCORES
    full_t = np.empty((H, T), np.float32)
    for c in range(NCORES):
        ot = results[c]["out_t"]
        for ch in range(NCH):
            g0 = ch * TCH + c * TSC
            full_t[:, g0:g0 + TSC] = ot[:, ch * TSC:(ch + 1) * TSC]
    return np.ascontiguousarray(full_t.T).reshape(B, S, H)


_NC_CACHE = {}


def _get_nc(cfg_key, cfg):
    if cfg_key not in _NC_CACHE:
        _NC_CACHE[cfg_key] = build_nc(cfg)
    return _NC_CACHE[cfg_key]


def run(inputs, cfg, **kwargs):
    nc = _get_nc(tuple(sorted(cfg.items())), cfg)
    in_maps = prepare_inputs(inputs, cfg)
    res = run_bass_kernel_spmd(nc, in_maps,
                               core_ids=list(range(cfg["NC"])), **kwargs)
    return assemble_output(res.results, cfg), res


def kernel(**inputs) -> np.ndarray:
    out, _ = run(inputs, FULL_CFG)
    return out



# revision 2
# speedup vs baseline: 40.3049x; 40.3049x over previous
"""Tensor-parallel LlamaDecoderLayer forward on 8 Trainium2 NeuronCores. v3.

Self-contained: shards FULL inputs across 8 cores inside kernel(), runs the
Bass kernel via run_bass_kernel_spmd, and reassembles the FULL output.

Sharding (per the TP hint): 4 q-heads + 1 kv-head per core; o/down
row-sharded; gate/up column-sharded over the (padded) intermediate dim.

v3 changes vs v2:
- the per-chunk AllReduce of o-proj partials (224.7us each in the collective
  cost model: out_bytes/40GBps * 1.875 + 15us) is replaced by
  ReduceScatter (28us) -> tiny local add h_slice = ro + x (own 64 tokens)
  -> AllGather of the h slices (120us). Same wire semantics, ~40% less
  serial time on the collective device, and the residual h never rides a
  second collective: the final output is rsm + ro + x computed from the
  RS outputs directly.
- norm2 is computed per-core on the gathered h (same vector work as v2).
- the MLP down-proj no longer folds in the o-proj partial (the residual
  is reconstructed at the end from ro + x), dropping 128 opart DMAs.
"""

import numpy as np
import ml_dtypes

import concourse.bacc as bacc
import concourse.bass as bass
import concourse.mybir as mybir
import concourse.tile as tile
from concourse.bass_utils import run_bass_kernel_spmd
from concourse.masks import make_identity

AF = mybir.ActivationFunctionType
ALU = mybir.AluOpType
DT = mybir.dt
BF16 = ml_dtypes.bfloat16

FULL_CFG = dict(H=4096, S=1024, B=2, NQH=4, D=128, IC=1408, NC=8, EPS=1e-5)


def build_nc(cfg):
    H, S, B = cfg["H"], cfg["S"], cfg["B"]
    NQH, D, IC = cfg["NQH"], cfg["D"], cfg["IC"]
    NCORES, EPS = cfg["NC"], cfg["EPS"]
    T = B * S
    KT = H // 128           # hidden-dim k tiles
    FQK = NQH + 1           # q tiles + 1 k tile
    NF = FQK + 1            # + v tile
    KI = IC // 128          # intermediate k tiles (per-core shard)
    MH = H // 128           # output feature tiles
    TCH = 512               # token chunk (collective + matmul granularity)
    NCH = T // TCH          # 4 chunks; chunk ch = (batch ch//2, half ch%2)
    TSC = TCH // NCORES     # per-rank token shard within a chunk (64)
    TQ = TCH                # attention query chunk
    DIAG = TQ // 128        # diagonal (masked) kt blocks per query chunk
    SBK = S // 128          # seq kt blocks per batch
    HD = D // 2
    sm_scale = float(1.0 / np.sqrt(D))
    f32, bf16 = DT.float32, DT.bfloat16
    rg = [list(range(NCORES))]

    nc = bacc.Bacc("TRN2", target_bir_lowering=False, debug=False,
                   num_devices=NCORES)

    xt = nc.dram_tensor("xt", [H, T], bf16, kind="ExternalInput")
    xs = nc.dram_tensor("xs", [H, NCH * TSC], f32, kind="ExternalInput")
    xsb = nc.dram_tensor("xsb", [H, NCH * TSC], bf16, kind="ExternalInput")
    wqkv = nc.dram_tensor("wqkv", [H, NF * 128], bf16, kind="ExternalInput")
    wo = nc.dram_tensor("wo", [NQH * D, H], bf16, kind="ExternalInput")
    wgu = nc.dram_tensor("wgu", [2 * KI, 128, KT, 128], bf16,
                         kind="ExternalInput")
    wdn = nc.dram_tensor("wdn", [MH, 128, KI, 128], bf16,
                         kind="ExternalInput")
    cs = nc.dram_tensor("cs", [D, T], f32, kind="ExternalInput")
    sn = nc.dram_tensor("sn", [D, T], f32, kind="ExternalInput")
    msk = nc.dram_tensor("msk", [DIAG, 128, TQ], bf16, kind="ExternalInput")
    out_t = nc.dram_tensor("out_t", [H, NCH * TSC], f32,
                           kind="ExternalOutput")

    wqkv_r = wqkv.ap().rearrange("(ko p) f -> p ko f", p=128)
    wo_r = wo.ap().rearrange("(ko p) f -> p ko f", p=128)
    xs_r = xs.ap().rearrange("(k p) t -> p k t", p=128)
    xsb_r = xsb.ap().rearrange("(k p) t -> p k t", p=128)
    out_r = out_t.ap().rearrange("(k p) t -> p k t", p=128)

    with tile.TileContext(nc, num_cores=NCORES) as tc:
        with (
            tc.tile_pool(name="misc", bufs=1) as miscp,
            tc.tile_pool(name="small", bufs=2) as smallp,
            tc.tile_pool(name="hd", bufs=2) as hdp,
            tc.tile_pool(name="dram", bufs=1, space="DRAM") as dramp,
            tc.tile_pool(name="ps", bufs=1, space="PSUM") as psp,
        ):
            ones_bf = miscp.tile([128, 1], bf16, tag="ones_bf")
            nc.gpsimd.memset(ones_bf, 1.0)
            eps_col = miscp.tile([128, 1], f32, tag="eps_col")
            nc.gpsimd.memset(eps_col, EPS)
            # ones row for PE-side partition broadcast (out = ones.T @ row).
            # gpsimd must stay free of compute: collective_compute blocks the
            # issuing engine queue until the ring completes, so any gpsimd op
            # queued after an in-flight collective would stall its consumers.
            ones_row = miscp.tile([1, 128], f32, tag="ones_row")
            nc.gpsimd.memset(ones_row, 1.0)

            def bcast(row, tag):
                ps_bc = psp.tile([128, row.shape[-1]], f32, tag="trbc",
                                 bufs=1, name=f"bc_{tag}")
                nc.tensor.matmul(ps_bc, ones_row, row, start=True, stop=True,
                                 skip_group_check=True)
                rb = smallp.tile([128, row.shape[-1]], f32, tag=tag)
                nc.scalar.copy(rb, ps_bc)
                return rb

            # o-proj partial slabs -> RS -> own-token o sums
            b_o = [dramp.tile([NCORES, H, TSC], bf16, tag=f"b_o{c}",
                              name=f"b_o{c}") for c in range(NCH)]
            ro = [dramp.tile([H, TSC], bf16, tag=f"ro{c}",
                             name=f"ro{c}") for c in range(NCH)]
            # h slices -> AG -> full h per chunk
            agin = [dramp.tile([H, TSC], bf16, tag=f"agin{c}",
                               name=f"agin{c}") for c in range(NCH)]
            ago = [dramp.tile([NCORES, H, TSC], bf16, tag=f"ago{c}",
                              name=f"ago{c}", addr_space="Shared")
                   for c in range(NCH)]
            # mlp partial slabs -> RS -> own-token mlp sums
            b2 = [dramp.tile([NCORES, H, TSC], bf16, tag=f"b2_{c}",
                             name=f"b2_{c}") for c in range(NCH)]
            rsm = [dramp.tile([H, TSC], bf16, tag=f"rsm{c}",
                              name=f"rsm{c}") for c in range(NCH)]

            def hdist(ch):
                # own-token h slice: h = o_sum + x, then AllGather the slices
                ro_sb = hdp.tile([128, KT, TSC], bf16, tag="ro_sb", bufs=1)
                nc.gpsimd.dma_start(
                    ro_sb, ro[ch].rearrange("(k p) t -> p k t", p=128))
                xss = hdp.tile([128, KT, TSC], bf16, tag="xss", bufs=1)
                nc.gpsimd.dma_start(
                    xss, xsb_r[:, :, ch * TSC:(ch + 1) * TSC])
                nc.gpsimd.tensor_tensor(ro_sb, ro_sb, xss, ALU.add)
                nc.gpsimd.dma_start(
                    agin[ch].rearrange("(k p) t -> p k t", p=128), ro_sb)
                nc.gpsimd.collective_compute(
                    "AllGather", ALU.bypass, replica_groups=rg,
                    ins=[agin[ch].opt()], outs=[ago[ch].opt()])

            # ============ scope 1: qkv + attention + o-proj + RS ============
            with (
                tc.tile_pool(name="bigAB", bufs=1) as bigp,
                tc.tile_pool(name="strAB", bufs=4) as strp,
                tc.tile_pool(name="tmpAB", bufs=3) as tmpp,
            ):
                ident = bigp.tile([128, 128], f32, tag="ident")
                make_identity(nc, ident)
                cs_sb = bigp.tile([128, T], f32, tag="cs")
                nc.gpsimd.dma_start(cs_sb, cs.ap())
                sn_sb = bigp.tile([128, T], f32, tag="sn")
                nc.gpsimd.dma_start(sn_sb, sn.ap())
                msk_sb = bigp.tile([128, DIAG, TQ], bf16, tag="msk")
                for j in range(DIAG):
                    nc.gpsimd.dma_start(msk_sb[:, j, :], msk.ap()[j])

                wqkv_sb = bigp.tile([128, KT, NF * 128], bf16, tag="wqkv")
                wo_sb = bigp.tile([128, NQH, H], bf16, tag="wo")
                qk_bf = bigp.tile([128, FQK, T], bf16, tag="qk")
                v_tok = bigp.tile([128, T // 128, 128], bf16, tag="vtok")
                attn_sb = bigp.tile([128, NQH, T], bf16, tag="attn")

                # ---- phase A: rmsnorm-folded qkv + rope (per token chunk) --
                def phase_a(tci):
                    t0 = tci * TCH
                    ps_qkv = [psp.tile([128, TCH], f32, tag="acc", bufs=6,
                                       name=f"ps_qkv{f}")
                              for f in range(NF)]
                    ps_ss = psp.tile([1, TCH], f32, tag="ss", bufs=1)
                    for k in range(KT):
                        if tci == 0:
                            nc.sync.dma_start(wqkv_sb[:, k, :],
                                              wqkv_r[:, k, :])
                        xtile = strp.tile([128, TCH], bf16, tag="x_in",
                                          bufs=4)
                        nc.sync.dma_start(
                            xtile, xt.ap()[k * 128:(k + 1) * 128,
                                           t0:t0 + TCH])
                        sq = strp.tile([128, TCH], bf16, tag="sq", bufs=2)
                        nc.scalar.square(sq, xtile)
                        nc.tensor.matmul(ps_ss, ones_bf, sq,
                                         start=(k == 0), stop=(k == KT - 1),
                                         skip_group_check=True)
                        for f in range(NF):
                            nc.tensor.matmul(
                                ps_qkv[f],
                                wqkv_sb[:, k, f * 128:(f + 1) * 128],
                                xtile, start=(k == 0), stop=(k == KT - 1),
                                skip_group_check=True)
                    ms = smallp.tile([1, TCH], f32, tag="ms")
                    nc.scalar.activation(ms, ps_ss, AF.Sqrt,
                                         bias=eps_col[0:1, :], scale=1.0 / H)
                    rr = smallp.tile([1, TCH], f32, tag="rr")
                    nc.vector.reciprocal(rr, ms)
                    rb = bcast(rr, "rb_a")
                    for f in range(FQK):
                        qf = tmpp.tile([128, TCH], f32, tag="qf", bufs=3)
                        nc.vector.tensor_tensor(qf, ps_qkv[f], rb, ALU.mult)
                        rot = tmpp.tile([128, TCH], f32, tag="rot", bufs=2)
                        nc.scalar.copy(rot[0:HD, :], qf[HD:D, :])
                        nc.scalar.copy(rot[HD:D, :], qf[0:HD, :])
                        nc.vector.tensor_tensor(rot, rot,
                                                sn_sb[:, t0:t0 + TCH],
                                                ALU.mult)
                        nc.vector.tensor_tensor(qf, qf,
                                                cs_sb[:, t0:t0 + TCH],
                                                ALU.mult)
                        nc.vector.tensor_tensor(qk_bf[:, f, t0:t0 + TCH],
                                                qf, rot, ALU.add)
                    vt = tmpp.tile([128, TCH], f32, tag="vt", bufs=2)
                    nc.vector.tensor_tensor(vt, ps_qkv[NF - 1], rb, ALU.mult)
                    ps_tr = psp.tile([128, TCH], f32, tag="trbc", bufs=1)
                    for j in range(TCH // 128):
                        nc.tensor.transpose(
                            ps_tr[:, j * 128:(j + 1) * 128],
                            vt[:, j * 128:(j + 1) * 128], ident)
                        nc.scalar.copy(v_tok[:, (t0 // 128) + j, :],
                                       ps_tr[:, j * 128:(j + 1) * 128])

                # ---- phase B+C per chunk: attention, o-proj, RS ----
                def phase_bc(ch):
                    b, qc = ch // 2, ch % 2
                    qt0 = qc * TQ
                    nkt = (qt0 + TQ) // 128
                    diag0 = nkt - DIAG
                    for h in range(NQH):
                        ps_o = psp.tile([128, TQ], f32, tag="acc", bufs=6)
                        ps_den = psp.tile([1, TQ], f32, tag="ss", bufs=1)
                        for kt in range(nkt):
                            ps_s = psp.tile([128, TQ], f32, tag="acc",
                                            bufs=6)
                            nc.tensor.matmul(
                                ps_s,
                                qk_bf[:, NQH, b * S + kt * 128:
                                      b * S + (kt + 1) * 128],
                                qk_bf[:, h, b * S + qt0:b * S + qt0 + TQ],
                                start=True, stop=True,
                                skip_group_check=True)
                            e_sb = tmpp.tile([128, TQ], bf16, tag="e_sb",
                                             bufs=3)
                            nc.scalar.activation(e_sb, ps_s, AF.Exp,
                                                 scale=sm_scale)
                            if kt >= diag0:
                                nc.vector.tensor_tensor(
                                    e_sb, e_sb, msk_sb[:, kt - diag0, :],
                                    ALU.mult)
                            nc.tensor.matmul(
                                ps_o, v_tok[:, b * SBK + kt, :], e_sb,
                                start=(kt == 0), stop=(kt == nkt - 1),
                                skip_group_check=True)
                            nc.tensor.matmul(
                                ps_den, ones_bf, e_sb,
                                start=(kt == 0), stop=(kt == nkt - 1),
                                skip_group_check=True)
                        rden = smallp.tile([1, TQ], f32, tag="rden")
                        nc.vector.reciprocal(rden, ps_den)
                        rbd = bcast(rden, "rbd")
                        nc.vector.tensor_tensor(
                            attn_sb[:, h, b * S + qt0:b * S + qt0 + TQ],
                            ps_o, rbd, ALU.mult)

                    # o-proj for this chunk, then its ReduceScatter
                    for m in range(MH):
                        ps = psp.tile([128, TCH], f32, tag="acc", bufs=6)
                        for kh in range(NQH):
                            nc.tensor.matmul(
                                ps, wo_sb[:, kh, m * 128:(m + 1) * 128],
                                attn_sb[:, kh,
                                        b * S + qt0:b * S + qt0 + TCH],
                                start=(kh == 0), stop=(kh == NQH - 1))
                        ob = tmpp.tile([128, TCH], bf16, tag="o_bf", bufs=3)
                        # vector, not scalar: keeps the scalar queue clear so
                        # the post-RS reads can issue promptly
                        nc.vector.tensor_scalar_mul(ob, ps, 1.0)
                        nc.sync.dma_start(
                            b_o[ch][:, m * 128:(m + 1) * 128, :]
                            .rearrange("r p t -> p r t"),
                            ob.rearrange("p (r t) -> p r t", r=NCORES))
                    nc.gpsimd.collective_compute(
                        "ReduceScatter", ALU.add, replica_groups=rg,
                        ins=[b_o[ch].opt()], outs=[ro[ch].opt()])

                # batch 0 first so RS0/RS1 fly while batch 1's qkv computes
                phase_a(0)
                phase_a(1)
                for kh in range(NQH):
                    nc.sync.dma_start(wo_sb[:, kh, :], wo_r[:, kh, :])
                phase_bc(0)
                phase_bc(1)
                hdist(0)
                phase_a(2)
                phase_a(3)
                hdist(1)
                phase_bc(2)
                hdist(2)
                phase_bc(3)
                hdist(3)

            # ============ scope 2: norm2 + MLP + RS per chunk ============
            with (
                tc.tile_pool(name="bigCD", bufs=1) as bigp2,
                tc.tile_pool(name="strCD", bufs=4) as strp2,
                tc.tile_pool(name="tmpCD", bufs=3) as tmpp2,
            ):
                for ch in range(NCH):
                    ago_r = ago[ch].rearrange("r (k p) t -> p k r t", p=128)
                    mi = bigp2.tile([128, KT, NCORES, TSC], bf16, tag="mi",
                                    bufs=2)
                    ps_ss2 = psp.tile([1, TCH], f32, tag="ss", bufs=1)
                    for k in range(KT):
                        # gathered-h reads ride the scalar queue: they wait on
                        # the AllGather, and a sync-queue wait would
                        # head-of-line block weight prefetch DMAs behind it
                        nc.scalar.dma_start(mi[:, k], ago_r[:, k])
                        sqh = strp2.tile([128, TCH], bf16, tag="sqh", bufs=3)
                        nc.scalar.square(
                            sqh, mi[:, k].rearrange("p r t -> p (r t)"))
                        nc.tensor.matmul(ps_ss2, ones_bf, sqh,
                                         start=(k == 0), stop=(k == KT - 1),
                                         skip_group_check=True)
                    ms2 = smallp.tile([1, TCH], f32, tag="ms2")
                    nc.scalar.activation(ms2, ps_ss2, AF.Sqrt,
                                         bias=eps_col[0:1, :], scale=1.0 / H)
                    rr2 = smallp.tile([1, TCH], f32, tag="rr2")
                    nc.vector.reciprocal(rr2, ms2)
                    rb2 = bcast(rr2, "rb2")
                    rb2_r = rb2.rearrange("p (r t) -> p r t", r=NCORES)
                    for k in range(KT):
                        nc.vector.tensor_tensor(mi[:, k], mi[:, k],
                                                rb2_r, ALU.mult)

                    act = bigp2.tile([128, KI, TCH], bf16, tag="act", bufs=2)
                    for fi in range(KI):
                        wg = strp2.tile([128, KT, 128], bf16, tag="wg",
                                        bufs=2)
                        nc.sync.dma_start(wg, wgu.ap()[fi])
                        wu = strp2.tile([128, KT, 128], bf16, tag="wu",
                                        bufs=2)
                        nc.sync.dma_start(wu, wgu.ap()[KI + fi])
                        ps_g = psp.tile([128, TCH], f32, tag="acc", bufs=6)
                        ps_u = psp.tile([128, TCH], f32, tag="acc", bufs=6)
                        for k in range(KT):
                            nc.tensor.matmul(ps_g, wg[:, k, :], mi[:, k],
                                             start=(k == 0),
                                             stop=(k == KT - 1),
                                             skip_group_check=True)
                        for k in range(KT):
                            nc.tensor.matmul(ps_u, wu[:, k, :], mi[:, k],
                                             start=(k == 0),
                                             stop=(k == KT - 1),
                                             skip_group_check=True)
                        sg = tmpp2.tile([128, TCH], f32, tag="sg", bufs=2)
                        nc.scalar.activation(sg, ps_g, AF.Silu)
                        nc.vector.tensor_tensor(act[:, fi, :], sg, ps_u,
                                                ALU.mult)

                    # down-proj; the residual is reconstructed at the end
                    # from ro + x, so the slab carries only mlp partials
                    for m in range(MH):
                        wds = strp2.tile([128, KI, 128], bf16, tag="wd",
                                         bufs=4)
                        nc.sync.dma_start(wds, wdn.ap()[m])
                        ps = psp.tile([128, TCH], f32, tag="acc", bufs=6)
                        for k2 in range(KI):
                            nc.tensor.matmul(ps, wds[:, k2, :],
                                             act[:, k2, :],
                                             start=(k2 == 0),
                                             stop=(k2 == KI - 1))
                        ob2 = tmpp2.tile([128, TCH], bf16, tag="d_bf",
                                         bufs=3)
                        nc.vector.tensor_scalar_mul(ob2, ps, 1.0)
                        nc.sync.dma_start(
                            b2[ch][:, m * 128:(m + 1) * 128, :]
                            .rearrange("r p t -> p r t"),
                            ob2.rearrange("p (r t) -> p r t", r=NCORES))
                    nc.gpsimd.collective_compute(
                        "ReduceScatter", ALU.add, replica_groups=rg,
                        ins=[b2[ch].opt()], outs=[rsm[ch].opt()])

                # ---- final: out = rsm + ro + x on own token shards ----
                for ch in range(NCH):
                    ro_sb2 = strp2.tile([128, KT, TSC], bf16, tag="ro_t",
                                        bufs=1)
                    nc.gpsimd.dma_start(
                        ro_sb2, ro[ch].rearrange("(k p) t -> p k t", p=128))
                    xst = strp2.tile([128, KT, TSC], f32, tag="xs_t", bufs=1)
                    nc.gpsimd.dma_start(
                        xst, xs_r[:, :, ch * TSC:(ch + 1) * TSC])
                    rsm_sb = strp2.tile([128, KT, TSC], bf16, tag="rsm_t",
                                        bufs=1)
                    nc.gpsimd.dma_start(
                        rsm_sb, rsm[ch].rearrange("(k p) t -> p k t", p=128))
                    t1 = tmpp2.tile([128, KT, TSC], f32, tag="t1", bufs=1)
                    nc.gpsimd.tensor_tensor(t1, rsm_sb, ro_sb2, ALU.add)
                    nc.gpsimd.tensor_tensor(t1, t1, xst, ALU.add)
                    nc.gpsimd.dma_start(
                        out_r[:, :, ch * TSC:(ch + 1) * TSC], t1)

    nc.compile()
    return nc


def prepare_inputs(inputs, cfg):
    """Full np inputs -> per-core in_maps (host-side sharding/prep)."""
    H, S, B = cfg["H"], cfg["S"], cfg["B"]
    NQH, D, IC = cfg["NQH"], cfg["D"], cfg["IC"]
    NCORES = cfg["NC"]
    T = B * S
    KT = H // 128
    KI = IC // 128
    MH = H // 128
    TCH = 512
    NCH = T // TCH
    TSC = TCH // NCORES
    TQ = TCH
    DIAG = TQ // 128
    f4 = np.float32

    x = np.asarray(inputs["x"], f4).reshape(T, H)
    xt_f = np.ascontiguousarray(x.T)                    # [H, T] f32
    xt = xt_f.astype(BF16)
    cos = np.asarray(inputs["cos"], f4)                 # [S, D]
    sin = np.asarray(inputs["sin"], f4)
    cs = np.ascontiguousarray(np.tile(cos.T, (1, B)))   # [D, T]
    sn_s = sin.T.copy()
    sn_s[: D // 2] *= -1.0                              # sign-folded rot_half
    sn = np.ascontiguousarray(np.tile(sn_s, (1, B)))
    anw = np.asarray(inputs["attn_norm_w"], f4)
    fnw = np.asarray(inputs["ffn_norm_w"], f4)
    wq = np.asarray(inputs["wq"], f4) * anw[:, None]
    wk = np.asarray(inputs["wk"], f4) * anw[:, None]
    wv = np.asarray(inputs["wv"], f4) * anw[:, None]
    wo = np.asarray(inputs["wo"], f4)
    wg = np.asarray(inputs["w_gate"], f4) * fnw[:, None]
    wu = np.asarray(inputs["w_up"], f4) * fnw[:, None]
    wd = np.asarray(inputs["w_down"], f4)
    I_full = wg.shape[1]
    I_pad = NCORES * IC
    if I_pad > I_full:
        pad = I_pad - I_full
        wg = np.pad(wg, ((0, 0), (0, pad)))
        wu = np.pad(wu, ((0, 0), (0, pad)))
        wd = np.pad(wd, ((0, pad), (0, 0)))

    # causal masks, transposed layout: keep (j*128 + kt) <= q
    kt_i = np.arange(128)[:, None]
    q_i = np.arange(TQ)[None, :]
    msk = np.stack([(j * 128 + kt_i <= q_i) for j in range(DIAG)]
                   ).astype(BF16)

    # xs: own 64-token slices per chunk (f32 residual path)
    qd, kvd = NQH * D, D
    in_maps = []
    for c in range(NCORES):
        wqkv = np.concatenate([
            wq[:, c * qd:(c + 1) * qd],
            wk[:, c * kvd:(c + 1) * kvd],
            wv[:, c * kvd:(c + 1) * kvd]], axis=1).astype(BF16)
        gu = np.concatenate([
            wg[:, c * IC:(c + 1) * IC],
            wu[:, c * IC:(c + 1) * IC]], axis=1)       # [H, 2*IC]
        wgu_s = np.ascontiguousarray(
            gu.reshape(KT, 128, 2 * KI, 128).transpose(2, 1, 0, 3)
        ).astype(BF16)                                  # [2KI, 128, KT, 128]
        dn = wd[c * IC:(c + 1) * IC, :]                 # [IC, H]
        wdn_s = np.ascontiguousarray(
            dn.reshape(KI, 128, MH, 128).transpose(2, 1, 0, 3)
        ).astype(BF16)                                  # [MH, 128, KI, 128]
        xs = np.empty((H, NCH * TSC), f4)
        for ch in range(NCH):
            g0 = ch * TCH + c * TSC
            xs[:, ch * TSC:(ch + 1) * TSC] = xt_f[:, g0:g0 + TSC]
        xs_c = np.ascontiguousarray(xs)
        in_maps.append({
            "xt": xt,
            "xs": xs_c,
            "xsb": xs_c.astype(BF16),
            "wqkv": np.ascontiguousarray(wqkv),
            "wo": np.ascontiguousarray(wo[c * qd:(c + 1) * qd, :]
                                       ).astype(BF16),
            "wgu": wgu_s,
            "wdn": wdn_s,
            "cs": cs, "sn": sn, "msk": msk,
        })
    return in_maps


def assemble_output(results, cfg):
    H, S, B, NCORES = cfg["H"], cfg["S"], cfg["B"], cfg["NC"]
    T = B * S
    TCH = 512
    NCH = T // TCH
    TSC = TCH // NCORES
    full_t = np.empty((H, T), np.float32)
    for c in range(NCORES):
        ot = results[c]["out_t"]
        for ch in range(NCH):
            g0 = ch * TCH + c * TSC
            full_t[:, g0:g0 + TSC] = ot[:, ch * TSC:(ch + 1) * TSC]
    return np.ascontiguousarray(full_t.T).reshape(B, S, H)


_NC_CACHE = {}


def _get_nc(cfg_key, cfg):
    if cfg_key not in _NC_CACHE:
        _NC_CACHE[cfg_key] = build_nc(cfg)
    return _NC_CACHE[cfg_key]


def run(inputs, cfg, **kwargs):
    nc = _get_nc(tuple(sorted(cfg.items())), cfg)
    in_maps = prepare_inputs(inputs, cfg)
    res = run_bass_kernel_spmd(nc, in_maps,
                               core_ids=list(range(cfg["NC"])), **kwargs)
    return assemble_output(res.results, cfg), res


def kernel(**inputs) -> np.ndarray:
    out, _ = run(inputs, FULL_CFG)
    return out


# revision 3
# speedup vs baseline: 47.6420x; 1.1820x over previous
"""Tensor-parallel LlamaDecoderLayer forward on 8 Trainium2 NeuronCores. v3.

Self-contained: shards FULL inputs across 8 cores inside kernel(), runs the
Bass kernel via run_bass_kernel_spmd, and reassembles the FULL output.

Sharding (per the TP hint): 4 q-heads + 1 kv-head per core; o/down
row-sharded; gate/up column-sharded over the (padded) intermediate dim.

v3 changes vs v2:
- the per-chunk AllReduce of o-proj partials (224.7us each in the collective
  cost model: out_bytes/40GBps * 1.875 + 15us) is replaced by
  ReduceScatter (28us) -> tiny local add h_slice = ro + x (own 64 tokens)
  -> AllGather of the h slices (120us). Same wire semantics, ~40% less
  serial time on the collective device, and the residual h never rides a
  second collective: the final output is rsm + ro + x computed from the
  RS outputs directly.
- norm2 is computed per-core on the gathered h (same vector work as v2).
- the MLP down-proj no longer folds in the o-proj partial (the residual
  is reconstructed at the end from ro + x), dropping 128 opart DMAs.
"""

import numpy as np
import ml_dtypes

import concourse.bacc as bacc
import concourse.bass as bass
import concourse.mybir as mybir
import concourse.tile as tile
from concourse.bass_utils import run_bass_kernel_spmd
from concourse.masks import make_identity

AF = mybir.ActivationFunctionType
ALU = mybir.AluOpType
DT = mybir.dt
BF16 = ml_dtypes.bfloat16

FULL_CFG = dict(H=4096, S=1024, B=2, NQH=4, D=128, IC=1408, NC=8, EPS=1e-5)


def build_nc(cfg):
    H, S, B = cfg["H"], cfg["S"], cfg["B"]
    NQH, D, IC = cfg["NQH"], cfg["D"], cfg["IC"]
    NCORES, EPS = cfg["NC"], cfg["EPS"]
    T = B * S
    KT = H // 128           # hidden-dim k tiles
    FQK = NQH + 1           # q tiles + 1 k tile
    NF = FQK + 1            # + v tile
    KI = IC // 128          # intermediate k tiles (per-core shard)
    MH = H // 128           # output feature tiles
    TCH = 512               # token chunk (collective + matmul granularity)
    NCH = T // TCH          # 4 chunks; chunk ch = (batch ch//2, half ch%2)
    TSC = TCH // NCORES     # per-rank token shard within a chunk (64)
    TQ = TCH                # attention query chunk
    DIAG = TQ // 128        # diagonal (masked) kt blocks per query chunk
    SBK = S // 128          # seq kt blocks per batch
    HD = D // 2
    sm_scale = float(1.0 / np.sqrt(D))
    f32, bf16 = DT.float32, DT.bfloat16
    rg = [list(range(NCORES))]

    nc = bacc.Bacc("TRN2", target_bir_lowering=False, debug=False,
                   num_devices=NCORES)

    xt = nc.dram_tensor("xt", [H, T], bf16, kind="ExternalInput")
    xs = nc.dram_tensor("xs", [H, NCH * TSC], f32, kind="ExternalInput")
    xsb = nc.dram_tensor("xsb", [H, NCH * TSC], bf16, kind="ExternalInput")
    wqkv = nc.dram_tensor("wqkv", [H, NF * 128], bf16, kind="ExternalInput")
    wo = nc.dram_tensor("wo", [NQH * D, H], bf16, kind="ExternalInput")
    wgu = nc.dram_tensor("wgu", [2 * KI, 128, KT, 128], bf16,
                         kind="ExternalInput")
    wdn = nc.dram_tensor("wdn", [MH, 128, KI, 128], bf16,
                         kind="ExternalInput")
    cs = nc.dram_tensor("cs", [D, T], f32, kind="ExternalInput")
    sn = nc.dram_tensor("sn", [D, T], f32, kind="ExternalInput")
    msk = nc.dram_tensor("msk", [DIAG, 128, TQ], bf16, kind="ExternalInput")
    out_t = nc.dram_tensor("out_t", [H, NCH * TSC], f32,
                           kind="ExternalOutput")

    wqkv_r = wqkv.ap().rearrange("(ko p) f -> p ko f", p=128)
    wo_r = wo.ap().rearrange("(ko p) f -> p ko f", p=128)
    xs_r = xs.ap().rearrange("(k p) t -> p k t", p=128)
    xsb_r = xsb.ap().rearrange("(k p) t -> p k t", p=128)
    out_r = out_t.ap().rearrange("(k p) t -> p k t", p=128)

    with tile.TileContext(nc, num_cores=NCORES) as tc:
        with (
            tc.tile_pool(name="misc", bufs=1) as miscp,
            tc.tile_pool(name="small", bufs=2) as smallp,
            tc.tile_pool(name="hd", bufs=2) as hdp,
            tc.tile_pool(name="dram", bufs=1, space="DRAM") as dramp,
            tc.tile_pool(name="ps", bufs=1, space="PSUM") as psp,
        ):
            ones_bf = miscp.tile([128, 1], bf16, tag="ones_bf")
            nc.gpsimd.memset(ones_bf, 1.0)
            eps_col = miscp.tile([128, 1], f32, tag="eps_col")
            nc.gpsimd.memset(eps_col, EPS)
            # ones row for PE-side partition broadcast (out = ones.T @ row).
            # gpsimd must stay free of compute: collective_compute blocks the
            # issuing engine queue until the ring completes, so any gpsimd op
            # queued after an in-flight collective would stall its consumers.
            ones_row = miscp.tile([1, 128], f32, tag="ones_row")
            nc.gpsimd.memset(ones_row, 1.0)

            def bcast(row, tag):
                ps_bc = psp.tile([128, row.shape[-1]], f32, tag="trbc",
                                 bufs=1, name=f"bc_{tag}")
                nc.tensor.matmul(ps_bc, ones_row, row, start=True, stop=True,
                                 skip_group_check=True)
                rb = smallp.tile([128, row.shape[-1]], f32, tag=tag)
                nc.scalar.copy(rb, ps_bc)
                return rb

            # o-proj partial slabs -> RS -> own-token o sums
            b_o = [dramp.tile([NCORES, H, TSC], bf16, tag=f"b_o{c}",
                              name=f"b_o{c}") for c in range(NCH)]
            ro = [dramp.tile([H, TSC], bf16, tag=f"ro{c}",
                             name=f"ro{c}") for c in range(NCH)]
            # h slices -> AG -> full h per chunk
            agin = [dramp.tile([H, TSC], bf16, tag=f"agin{c}",
                               name=f"agin{c}") for c in range(NCH)]
            ago = [dramp.tile([NCORES, H, TSC], bf16, tag=f"ago{c}",
                              name=f"ago{c}", addr_space="Shared")
                   for c in range(NCH)]
            # mlp partial slabs -> RS -> own-token mlp sums
            b2 = [dramp.tile([NCORES, H, TSC], bf16, tag=f"b2_{c}",
                             name=f"b2_{c}") for c in range(NCH)]
            rsm = [dramp.tile([H, TSC], bf16, tag=f"rsm{c}",
                              name=f"rsm{c}") for c in range(NCH)]

            def hdist(ch):
                # own-token h slice: h = o_sum + x, then AllGather the slices
                ro_sb = hdp.tile([128, KT, TSC], bf16, tag="ro_sb", bufs=1)
                nc.gpsimd.dma_start(
                    ro_sb, ro[ch].rearrange("(k p) t -> p k t", p=128))
                xss = hdp.tile([128, KT, TSC], bf16, tag="xss", bufs=1)
                nc.gpsimd.dma_start(
                    xss, xsb_r[:, :, ch * TSC:(ch + 1) * TSC])
                nc.gpsimd.tensor_tensor(ro_sb, ro_sb, xss, ALU.add)
                nc.gpsimd.dma_start(
                    agin[ch].rearrange("(k p) t -> p k t", p=128), ro_sb)
                nc.gpsimd.collective_compute(
                    "AllGather", ALU.bypass, replica_groups=rg,
                    ins=[agin[ch].opt()], outs=[ago[ch].opt()])

            # ============ scope 1: qkv + attention + o-proj + RS ============
            with (
                tc.tile_pool(name="bigAB", bufs=1) as bigp,
                tc.tile_pool(name="strAB", bufs=4) as strp,
                tc.tile_pool(name="tmpAB", bufs=3) as tmpp,
            ):
                ident = bigp.tile([128, 128], f32, tag="ident")
                make_identity(nc, ident)
                cs_sb = bigp.tile([128, T], f32, tag="cs")
                nc.gpsimd.dma_start(cs_sb, cs.ap())
                sn_sb = bigp.tile([128, T], f32, tag="sn")
                nc.gpsimd.dma_start(sn_sb, sn.ap())
                msk_sb = bigp.tile([128, DIAG, TQ], bf16, tag="msk")
                for j in range(DIAG):
                    nc.gpsimd.dma_start(msk_sb[:, j, :], msk.ap()[j])

                wqkv_sb = bigp.tile([128, KT, NF * 128], bf16, tag="wqkv")
                wo_sb = bigp.tile([128, NQH, H], bf16, tag="wo")
                qk_bf = bigp.tile([128, FQK, T], bf16, tag="qk")
                v_tok = bigp.tile([128, T // 128, 128], bf16, tag="vtok")
                attn_sb = bigp.tile([128, NQH, T], bf16, tag="attn")

                # ---- phase A: rmsnorm-folded qkv + rope (per token chunk) --
                def phase_a(tci):
                    t0 = tci * TCH
                    ps_qkv = [psp.tile([128, TCH], f32, tag="acc", bufs=6,
                                       name=f"ps_qkv{f}")
                              for f in range(NF)]
                    ps_ss = psp.tile([1, TCH], f32, tag="ss", bufs=1)
                    for k in range(KT):
                        if tci == 0:
                            nc.sync.dma_start(wqkv_sb[:, k, :],
                                              wqkv_r[:, k, :])
                        xtile = strp.tile([128, TCH], bf16, tag="x_in",
                                          bufs=4)
                        nc.sync.dma_start(
                            xtile, xt.ap()[k * 128:(k + 1) * 128,
                                           t0:t0 + TCH])
                        sq = strp.tile([128, TCH], bf16, tag="sq", bufs=2)
                        nc.scalar.square(sq, xtile)
                        nc.tensor.matmul(ps_ss, ones_bf, sq,
                                         start=(k == 0), stop=(k == KT - 1),
                                         skip_group_check=True)
                        for f in range(NF):
                            nc.tensor.matmul(
                                ps_qkv[f],
                                wqkv_sb[:, k, f * 128:(f + 1) * 128],
                                xtile, start=(k == 0), stop=(k == KT - 1),
                                skip_group_check=True)
                    ms = smallp.tile([1, TCH], f32, tag="ms")
                    nc.scalar.activation(ms, ps_ss, AF.Sqrt,
                                         bias=eps_col[0:1, :], scale=1.0 / H)
                    rr = smallp.tile([1, TCH], f32, tag="rr")
                    nc.vector.reciprocal(rr, ms)
                    rb = bcast(rr, "rb_a")
                    for f in range(FQK):
                        qf = tmpp.tile([128, TCH], f32, tag="qf", bufs=3)
                        nc.vector.tensor_tensor(qf, ps_qkv[f], rb, ALU.mult)
                        rot = tmpp.tile([128, TCH], f32, tag="rot", bufs=2)
                        nc.scalar.copy(rot[0:HD, :], qf[HD:D, :])
                        nc.scalar.copy(rot[HD:D, :], qf[0:HD, :])
                        nc.vector.tensor_tensor(rot, rot,
                                                sn_sb[:, t0:t0 + TCH],
                                                ALU.mult)
                        nc.vector.tensor_tensor(qf, qf,
                                                cs_sb[:, t0:t0 + TCH],
                                                ALU.mult)
                        nc.vector.tensor_tensor(qk_bf[:, f, t0:t0 + TCH],
                                                qf, rot, ALU.add)
                    vt = tmpp.tile([128, TCH], f32, tag="vt", bufs=2)
                    nc.vector.tensor_tensor(vt, ps_qkv[NF - 1], rb, ALU.mult)
                    ps_tr = psp.tile([128, TCH], f32, tag="trbc", bufs=1)
                    for j in range(TCH // 128):
                        nc.tensor.transpose(
                            ps_tr[:, j * 128:(j + 1) * 128],
                            vt[:, j * 128:(j + 1) * 128], ident)
                        nc.scalar.copy(v_tok[:, (t0 // 128) + j, :],
                                       ps_tr[:, j * 128:(j + 1) * 128])

                # ---- phase B+C per chunk: attention, o-proj, RS ----
                def phase_bc(ch):
                    b, qc = ch // 2, ch % 2
                    qt0 = qc * TQ
                    nkt = (qt0 + TQ) // 128
                    diag0 = nkt - DIAG
                    for h in range(NQH):
                        ps_o = psp.tile([128, TQ], f32, tag="acc", bufs=6)
                        ps_den = psp.tile([1, TQ], f32, tag="ss", bufs=1)
                        for kt in range(nkt):
                            ps_s = psp.tile([128, TQ], f32, tag="acc",
                                            bufs=6)
                            nc.tensor.matmul(
                                ps_s,
                                qk_bf[:, NQH, b * S + kt * 128:
                                      b * S + (kt + 1) * 128],
                                qk_bf[:, h, b * S + qt0:b * S + qt0 + TQ],
                                start=True, stop=True,
                                skip_group_check=True)
                            e_sb = tmpp.tile([128, TQ], bf16, tag="e_sb",
                                             bufs=3)
                            nc.scalar.activation(e_sb, ps_s, AF.Exp,
                                                 scale=sm_scale)
                            if kt >= diag0:
                                nc.vector.tensor_tensor(
                                    e_sb, e_sb, msk_sb[:, kt - diag0, :],
                                    ALU.mult)
                            nc.tensor.matmul(
                                ps_o, v_tok[:, b * SBK + kt, :], e_sb,
                                start=(kt == 0), stop=(kt == nkt - 1),
                                skip_group_check=True)
                            nc.tensor.matmul(
                                ps_den, ones_bf, e_sb,
                                start=(kt == 0), stop=(kt == nkt - 1),
                                skip_group_check=True)
                        rden = smallp.tile([1, TQ], f32, tag="rden")
                        nc.vector.reciprocal(rden, ps_den)
                        rbd = bcast(rden, "rbd")
                        nc.vector.tensor_tensor(
                            attn_sb[:, h, b * S + qt0:b * S + qt0 + TQ],
                            ps_o, rbd, ALU.mult)

                    # o-proj for this chunk, then its ReduceScatter
                    for m in range(MH):
                        ps = psp.tile([128, TCH], f32, tag="acc", bufs=6)
                        for kh in range(NQH):
                            nc.tensor.matmul(
                                ps, wo_sb[:, kh, m * 128:(m + 1) * 128],
                                attn_sb[:, kh,
                                        b * S + qt0:b * S + qt0 + TCH],
                                start=(kh == 0), stop=(kh == NQH - 1))
                        ob = tmpp.tile([128, TCH], bf16, tag="o_bf", bufs=3)
                        # vector, not scalar: keeps the scalar queue clear so
                        # the post-RS reads can issue promptly
                        nc.vector.tensor_scalar_mul(ob, ps, 1.0)
                        nc.sync.dma_start(
                            b_o[ch][:, m * 128:(m + 1) * 128, :]
                            .rearrange("r p t -> p r t"),
                            ob.rearrange("p (r t) -> p r t", r=NCORES))
                    nc.gpsimd.collective_compute(
                        "ReduceScatter", ALU.add, replica_groups=rg,
                        ins=[b_o[ch].opt()], outs=[ro[ch].opt()])

                # batch 0 first so RS0/RS1 fly while batch 1's qkv computes
                phase_a(0)
                phase_a(1)
                for kh in range(NQH):
                    nc.sync.dma_start(wo_sb[:, kh, :], wo_r[:, kh, :])
                phase_bc(0)
                phase_bc(1)
                hdist(0)
                phase_a(2)
                phase_a(3)
                hdist(1)
                phase_bc(2)
                hdist(2)
                phase_bc(3)
                hdist(3)

            # ============ scope 2: norm2 + MLP + RS per chunk ============
            with (
                tc.tile_pool(name="bigCD", bufs=1) as bigp2,
                tc.tile_pool(name="strCD", bufs=4) as strp2,
                tc.tile_pool(name="tmpCD", bufs=3) as tmpp2,
            ):
                for ch in range(NCH):
                    ago_r = ago[ch].rearrange("r (k p) t -> p k r t", p=128)
                    mi = bigp2.tile([128, KT, NCORES, TSC], bf16, tag="mi",
                                    bufs=2)
                    ps_ss2 = psp.tile([1, TCH], f32, tag="ss", bufs=1)
                    for k in range(KT):
                        # gathered-h reads ride the scalar queue: they wait on
                        # the AllGather, and a sync-queue wait would
                        # head-of-line block weight prefetch DMAs behind it
                        nc.scalar.dma_start(mi[:, k], ago_r[:, k])
                        sqh = strp2.tile([128, TCH], bf16, tag="sqh", bufs=3)
                        mik = mi[:, k].rearrange("p r t -> p (r t)")
                        nc.vector.tensor_tensor(sqh, mik, mik, ALU.mult)
                        nc.tensor.matmul(ps_ss2, ones_bf, sqh,
                                         start=(k == 0), stop=(k == KT - 1),
                                         skip_group_check=True)
                    ms2 = smallp.tile([1, TCH], f32, tag="ms2")
                    nc.scalar.activation(ms2, ps_ss2, AF.Sqrt,
                                         bias=eps_col[0:1, :], scale=1.0 / H)
                    rr2 = smallp.tile([1, TCH], f32, tag="rr2")
                    nc.vector.reciprocal(rr2, ms2)
                    rb2 = bcast(rr2, "rb2")
                    rb2_r = rb2.rearrange("p (r t) -> p r t", r=NCORES)
                    for k in range(KT):
                        nc.vector.tensor_tensor(mi[:, k], mi[:, k],
                                                rb2_r, ALU.mult)

                    act = bigp2.tile([128, KI, TCH], bf16, tag="act", bufs=2)
                    for fi in range(KI):
                        wg = strp2.tile([128, KT, 128], bf16, tag="wg",
                                        bufs=2)
                        nc.sync.dma_start(wg, wgu.ap()[fi])
                        wu = strp2.tile([128, KT, 128], bf16, tag="wu",
                                        bufs=2)
                        nc.sync.dma_start(wu, wgu.ap()[KI + fi])
                        ps_g = psp.tile([128, TCH], f32, tag="acc", bufs=6)
                        ps_u = psp.tile([128, TCH], f32, tag="acc", bufs=6)
                        for k in range(KT):
                            nc.tensor.matmul(ps_g, wg[:, k, :], mi[:, k],
                                             start=(k == 0),
                                             stop=(k == KT - 1),
                                             skip_group_check=True)
                        for k in range(KT):
                            nc.tensor.matmul(ps_u, wu[:, k, :], mi[:, k],
                                             start=(k == 0),
                                             stop=(k == KT - 1),
                                             skip_group_check=True)
                        sg = tmpp2.tile([128, TCH], f32, tag="sg", bufs=2)
                        nc.scalar.activation(sg, ps_g, AF.Silu)
                        nc.vector.tensor_tensor(act[:, fi, :], sg, ps_u,
                                                ALU.mult)

                    # down-proj; the residual is reconstructed at the end
                    # from ro + x, so the slab carries only mlp partials
                    for m in range(MH):
                        wds = strp2.tile([128, KI, 128], bf16, tag="wd",
                                         bufs=4)
                        nc.sync.dma_start(wds, wdn.ap()[m])
                        ps = psp.tile([128, TCH], f32, tag="acc", bufs=6)
                        for k2 in range(KI):
                            nc.tensor.matmul(ps, wds[:, k2, :],
                                             act[:, k2, :],
                                             start=(k2 == 0),
                                             stop=(k2 == KI - 1))
                        ob2 = tmpp2.tile([128, TCH], bf16, tag="d_bf",
                                         bufs=3)
                        nc.vector.tensor_scalar_mul(ob2, ps, 1.0)
                        nc.sync.dma_start(
                            b2[ch][:, m * 128:(m + 1) * 128, :]
                            .rearrange("r p t -> p r t"),
                            ob2.rearrange("p (r t) -> p r t", r=NCORES))
                    nc.gpsimd.collective_compute(
                        "ReduceScatter", ALU.add, replica_groups=rg,
                        ins=[b2[ch].opt()], outs=[rsm[ch].opt()])

                # ---- final: out = rsm + ro + x on own token shards ----
                for ch in range(NCH):
                    ro_sb2 = strp2.tile([128, KT, TSC], bf16, tag="ro_t",
                                        bufs=1)
                    nc.gpsimd.dma_start(
                        ro_sb2, ro[ch].rearrange("(k p) t -> p k t", p=128))
                    xst = strp2.tile([128, KT, TSC], f32, tag="xs_t", bufs=1)
                    nc.gpsimd.dma_start(
                        xst, xs_r[:, :, ch * TSC:(ch + 1) * TSC])
                    rsm_sb = strp2.tile([128, KT, TSC], bf16, tag="rsm_t",
                                        bufs=1)
                    nc.gpsimd.dma_start(
                        rsm_sb, rsm[ch].rearrange("(k p) t -> p k t", p=128))
                    t1 = tmpp2.tile([128, KT, TSC], f32, tag="t1", bufs=1)
                    nc.gpsimd.tensor_tensor(t1, rsm_sb, ro_sb2, ALU.add)
                    nc.gpsimd.tensor_tensor(t1, t1, xst, ALU.add)
                    nc.gpsimd.dma_start(
                        out_r[:, :, ch * TSC:(ch + 1) * TSC], t1)

    nc.compile()
    return nc


def prepare_inputs(inputs, cfg):
    """Full np inputs -> per-core in_maps (host-side sharding/prep)."""
    H, S, B = cfg["H"], cfg["S"], cfg["B"]
    NQH, D, IC = cfg["NQH"], cfg["D"], cfg["IC"]
    NCORES = cfg["NC"]
    T = B * S
    KT = H // 128
    KI = IC // 128
    MH = H // 128
    TCH = 512
    NCH = T // TCH
    TSC = TCH // NCORES
    TQ = TCH
    DIAG = TQ // 128
    f4 = np.float32

    x = np.asarray(inputs["x"], f4).reshape(T, H)
    xt_f = np.ascontiguousarray(x.T)                    # [H, T] f32
    xt = xt_f.astype(BF16)
    cos = np.asarray(inputs["cos"], f4)                 # [S, D]
    sin = np.asarray(inputs["sin"], f4)
    cs = np.ascontiguousarray(np.tile(cos.T, (1, B)))   # [D, T]
    sn_s = sin.T.copy()
    sn_s[: D // 2] *= -1.0                              # sign-folded rot_half
    sn = np.ascontiguousarray(np.tile(sn_s, (1, B)))
    anw = np.asarray(inputs["attn_norm_w"], f4)
    fnw = np.asarray(inputs["ffn_norm_w"], f4)
    wq = np.asarray(inputs["wq"], f4) * anw[:, None]
    wk = np.asarray(inputs["wk"], f4) * anw[:, None]
    wv = np.asarray(inputs["wv"], f4) * anw[:, None]
    wo = np.asarray(inputs["wo"], f4)
    wg = np.asarray(inputs["w_gate"], f4) * fnw[:, None]
    wu = np.asarray(inputs["w_up"], f4) * fnw[:, None]
    wd = np.asarray(inputs["w_down"], f4)
    I_full = wg.shape[1]
    I_pad = NCORES * IC
    if I_pad > I_full:
        pad = I_pad - I_full
        wg = np.pad(wg, ((0, 0), (0, pad)))
        wu = np.pad(wu, ((0, 0), (0, pad)))
        wd = np.pad(wd, ((0, pad), (0, 0)))

    # causal masks, transposed layout: keep (j*128 + kt) <= q
    kt_i = np.arange(128)[:, None]
    q_i = np.arange(TQ)[None, :]
    msk = np.stack([(j * 128 + kt_i <= q_i) for j in range(DIAG)]
                   ).astype(BF16)

    # xs: own 64-token slices per chunk (f32 residual path)
    qd, kvd = NQH * D, D
    in_maps = []
    for c in range(NCORES):
        wqkv = np.concatenate([
            wq[:, c * qd:(c + 1) * qd],
            wk[:, c * kvd:(c + 1) * kvd],
            wv[:, c * kvd:(c + 1) * kvd]], axis=1).astype(BF16)
        gu = np.concatenate([
            wg[:, c * IC:(c + 1) * IC],
            wu[:, c * IC:(c + 1) * IC]], axis=1)       # [H, 2*IC]
        wgu_s = np.ascontiguousarray(
            gu.reshape(KT, 128, 2 * KI, 128).transpose(2, 1, 0, 3)
        ).astype(BF16)                                  # [2KI, 128, KT, 128]
        dn = wd[c * IC:(c + 1) * IC, :]                 # [IC, H]
        wdn_s = np.ascontiguousarray(
            dn.reshape(KI, 128, MH, 128).transpose(2, 1, 0, 3)
        ).astype(BF16)                                  # [MH, 128, KI, 128]
        xs = np.empty((H, NCH * TSC), f4)
        for ch in range(NCH):
            g0 = ch * TCH + c * TSC
            xs[:, ch * TSC:(ch + 1) * TSC] = xt_f[:, g0:g0 + TSC]
        xs_c = np.ascontiguousarray(xs)
        in_maps.append({
            "xt": xt,
            "xs": xs_c,
            "xsb": xs_c.astype(BF16),
            "wqkv": np.ascontiguousarray(wqkv),
            "wo": np.ascontiguousarray(wo[c * qd:(c + 1) * qd, :]
                                       ).astype(BF16),
            "wgu": wgu_s,
            "wdn": wdn_s,
            "cs": cs, "sn": sn, "msk": msk,
        })
    return in_maps


def assemble_output(results, cfg):
    H, S, B, NCORES = cfg["H"], cfg["S"], cfg["B"], cfg["NC"]
    T = B * S
    TCH = 512
    NCH = T // TCH
    TSC = TCH // NCORES
    full_t = np.empty((H, T), np.float32)
    for c in range(NCORES):
        ot = results[c]["out_t"]
        for ch in range(NCH):
            g0 = ch * TCH + c * TSC
            full_t[:, g0:g0 + TSC] = ot[:, ch * TSC:(ch + 1) * TSC]
    return np.ascontiguousarray(full_t.T).reshape(B, S, H)


_NC_CACHE = {}


def _get_nc(cfg_key, cfg):
    if cfg_key not in _NC_CACHE:
        _NC_CACHE[cfg_key] = build_nc(cfg)
    return _NC_CACHE[cfg_key]


def run(inputs, cfg, **kwargs):
    nc = _get_nc(tuple(sorted(cfg.items())), cfg)
    in_maps = prepare_inputs(inputs, cfg)
    res = run_bass_kernel_spmd(nc, in_maps,
                               core_ids=list(range(cfg["NC"])), **kwargs)
    return assemble_output(res.results, cfg), res


def kernel(**inputs) -> np.ndarray:
    out, _ = run(inputs, FULL_CFG)
    return out
